# revision 4
# baseline (speedup 1.0000x reference)
# Trainium2 Bass kernel for nn_DetectionLoss (B=32, N=25200, M=200, C=80).
#
# Strategy: pure data-parallel over batch (4 batches per core, 8 cores).
# The reference only reads pred_bbox[:, :M] and pred_cls[:, :M], so only
# those slices are shipped to the device. Each core computes per-partition
# partial sums of the loss terms; the host does the final (tiny) cross-core
# reduction and mean/lambda arithmetic in float64. The O(B*M) tails the
# host already gathers (positive-anchor softplus, picked class logits) are
# summed on the host in float64 - the device keeps every O(B*N) / O(B*M*C)
# reduction.
#
# Device input per core is ONE fp8 tensor big[128, 1952] so the whole
# transfer is 128 descriptors of 1952B (~2KB is the efficient descriptor
# size; the separate obj/cls/small tensors of the earlier kernel cost 328
# thin descriptors). It is DMA'd as 5 partition-slices, one per engine
# queue, so all 16 SDMA engines pull in parallel:
#   cols    0: 800  rows 0:126  all 4*25200 obj logits (flat, fp8e3)
#   cols  800:1440  rows 0:100  cls logits [p, a=8, c=80] (fp8e3)
#   cols 1440:1952  rows 0:50   pred|gt boxes as RAW f32 bytes
#                               [p, s=2, j=16, c=4] (bitcast view on device)
# Boxes stay f32 because the near-zero enclose/union denominators amplify
# input rounding; fp8 for the logits keeps ~1% per-element error that
# averages out over the 6400..100800-element means.
#
# On-device: one merged Exp over [128,1440] (obj+cls in one ACT pass),
# softplus tail as Ln(x*1+1) via the Ln activation's pre-bias with
# accum_out; lse as DVE reduce -> Ln with accum; GIoU on DVE from the
# bitcast f32 view; per-partition partials collapse to [1, 8] via a PE
# matmul with an iota-built 0/1 column so the output DMA is ONE descriptor.

import numpy as np

B, N, M, C = 32, 25200, 200, 80
NCORES = 8
BPC = B // NCORES          # 4 batches per core
P_OBJ, F_OBJ = 126, 800    # 4*25200 = 126*800 exactly
P_CLS = 100                # cls partitions: 8 anchors x 80 classes per row
W_CLS = 640
P_BOX = 50                 # box pair partitions: 16 pairs per row
NPAIR = 16
W_BIG = 1952               # 800 obj + 640 cls + 512 box bytes
EPS = 1e-7

_CACHED_NC = None


def _emit(nc, tc, mybir, big, out):
    f32 = mybir.dt.float32
    bf16 = mybir.dt.bfloat16
    Alu = mybir.AluOpType
    Act = mybir.ActivationFunctionType

    with tc.tile_pool(name="main", bufs=1) as pool:
        ACC = pool.tile([128, 8], f32, name="ACC")
        nc.vector.memset(ACC[:], 0.0)
        # Activation bias constants built in-block on DVE: the Bass preamble's
        # gpsimd const memsets gate the tile-enter dance, so registering our
        # own lets the (now-dead) preamble ones be stripped after compile.
        CB = pool.tile([128, 2], f32, name="CB")
        nc.vector.memset(CB[:, 0:1], 0.0)
        nc.vector.memset(CB[:, 1:2], 1.0)

        BIG = pool.tile([128, W_BIG], mybir.dt.float8e3, name="BIG")
        # One DMA slice per DMA-capable engine queue (SP, Pool, Activation).
        # Rows 0:50 (the box bytes) ride the Sync queue alone so the DVE
        # GIoU chain depends on a single early slice.
        nc.sync.dma_start(out=BIG[0:50], in_=big.ap()[0:50])
        nc.gpsimd.dma_start(out=BIG[50:89], in_=big.ap()[50:89])
        nc.scalar.dma_start(out=BIG[89:128], in_=big.ap()[89:128])

        # 0/1 column selecting partitions 0:126 for the final PE collapse.
        W = pool.tile([128, 1], f32, name="W")
        IOT = pool.tile([128, 1], mybir.dt.int32, name="IOT")
        nc.gpsimd.iota(IOT[:], pattern=[[0, 1]], base=0, channel_multiplier=1)
        nc.vector.tensor_scalar(W[:], IOT[:], P_OBJ, None, op0=Alu.is_lt)

        # ---------------- one Exp pass over obj+cls ----------------
        # bf16 output: the 0.4% rounding on e^x averages out over the
        # 100800-element softplus sum and the 80-class lse sums.
        E = pool.tile([128, 1440], bf16, name="E")
        nc.scalar.activation(E[:], BIG[:, 0:1440], Act.Exp, bias=CB[:, 0:1])

        # ---------------- bbox GIoU term ----------------
        # boxes ship as f32 bytes inside the fp8 tile: the near-zero
        # enclose/union denominators must match the reference's f32 math.
        PB = BIG[0:P_BOX, 1440:1952].bitcast(f32).rearrange(
            "p (s j c) -> p s j c", s=2, c=4
        )
        cxcy = PB[:, :, :, 0:2]
        wh = PB[:, :, :, 2:4]
        C1 = pool.tile([P_BOX, 2, NPAIR, 2], f32, name="C1")
        C2 = pool.tile([P_BOX, 2, NPAIR, 2], f32, name="C2")
        nc.vector.scalar_tensor_tensor(C1[:], wh, -0.5, cxcy, Alu.mult, Alu.add)
        nc.vector.scalar_tensor_tensor(C2[:], wh, 0.5, cxcy, Alu.mult, Alu.add)
        I1 = pool.tile([P_BOX, NPAIR, 2], f32, name="I1")
        I2 = pool.tile([P_BOX, NPAIR, 2], f32, name="I2")
        E1 = pool.tile([P_BOX, NPAIR, 2], f32, name="E1")
        E2 = pool.tile([P_BOX, NPAIR, 2], f32, name="E2")
        nc.vector.tensor_tensor(I1[:], C1[:, 0], C1[:, 1], Alu.max)
        nc.vector.tensor_tensor(I2[:], C2[:, 0], C2[:, 1], Alu.min)
        nc.vector.tensor_tensor(E1[:], C1[:, 0], C1[:, 1], Alu.min)
        nc.vector.tensor_tensor(E2[:], C2[:, 0], C2[:, 1], Alu.max)
        ID = pool.tile([P_BOX, NPAIR, 2], f32, name="ID")
        IDr = pool.tile([P_BOX, NPAIR, 2], f32, name="IDr")
        ED = pool.tile([P_BOX, NPAIR, 2], f32, name="ED")
        nc.vector.tensor_sub(ID[:], I2[:], I1[:])
        nc.vector.tensor_relu(IDr[:], ID[:])
        nc.vector.tensor_sub(ED[:], E2[:], E1[:])
        inter = pool.tile([P_BOX, NPAIR], f32, name="inter")
        encl = pool.tile([P_BOX, NPAIR], f32, name="encl")
        nc.vector.tensor_mul(inter[:], IDr[:, :, 0], IDr[:, :, 1])
        nc.vector.tensor_mul(encl[:], ED[:, :, 0], ED[:, :, 1])
        A = pool.tile([P_BOX, 2, NPAIR], f32, name="A")
        nc.vector.tensor_mul(A[:], PB[:, :, :, 2], PB[:, :, :, 3])
        asum = pool.tile([P_BOX, NPAIR], f32, name="asum")
        nc.vector.tensor_add(asum[:], A[:, 0], A[:, 1])
        U = pool.tile([P_BOX, NPAIR], f32, name="U")
        nc.vector.scalar_tensor_tensor(U[:], inter[:], -1.0, asum[:],
                                       Alu.mult, Alu.add)
        # U+eps and encl+eps laid adjacently so one reciprocal covers both
        R = pool.tile([P_BOX, 2, NPAIR], f32, name="R")
        R2 = pool.tile([P_BOX, 2, NPAIR], f32, name="R2")
        nc.vector.tensor_scalar_add(R[:, 0], U[:], EPS)
        nc.vector.tensor_scalar_add(R[:, 1], encl[:], EPS)
        nc.vector.reciprocal(R2[:], R[:])
        # NOTE: tensor_tensor_reduce wedges the device (NRT_EXEC_UNIT_UNRECOVERABLE)
        # on this runtime; scalar_tensor_tensor's accum_out path works.
        t8a = pool.tile([P_BOX, NPAIR], f32, name="t8a")
        nc.vector.scalar_tensor_tensor(
            t8a[:], inter[:], 1.0, R2[:, 0], Alu.mult, Alu.mult,
            accum_out=ACC[0:P_BOX, 0:1],
        )
        EmU = pool.tile([P_BOX, NPAIR], f32, name="EmU")
        nc.vector.tensor_sub(EmU[:], encl[:], U[:])
        t8b = pool.tile([P_BOX, NPAIR], f32, name="t8b")
        nc.vector.scalar_tensor_tensor(
            t8b[:], EmU[:], 1.0, R2[:, 1], Alu.mult, Alu.mult,
            accum_out=ACC[0:P_BOX, 1:2],
        )

        # ---------------- objectness softplus tail ----------------
        # sum softplus(x): Ln with pre-activation bias=1.0 (out = ln(in*1+1))
        # over the Exp output, with accum_out - no elementwise +1 pass.
        Lg = pool.tile([P_OBJ, F_OBJ], bf16, name="Lg")
        nc.scalar.activation(Lg[:], E[0:P_OBJ, 0:F_OBJ], Act.Ln,
                             bias=CB[0:P_OBJ, 1:2],
                             accum_out=ACC[0:P_OBJ, 2:3])

        # ---------------- classification lse tail ----------------
        sums = pool.tile([P_CLS, 8], f32, name="sums")
        lse = pool.tile([P_CLS, 8], f32, name="lse")
        nc.vector.reduce_sum(
            out=sums[:],
            in_=E[0:P_CLS, 800:1440].rearrange("p (a c) -> p a c", c=C),
            axis=mybir.AxisListType.X,
        )
        nc.scalar.activation(lse[:], sums[:], Act.Ln,
                             bias=CB[0:P_CLS, 0:1],
                             accum_out=ACC[0:P_CLS, 3:4])

        # Collapse ACC [128,8] to [1,8] on the idle PE (sum over partitions
        # 0:126) so the output DMA is a single descriptor.
        from concourse.bass import MemorySpace
        PS, _ps_free = tc.tile([1, 8], f32, space=MemorySpace.PSUM, name="PS")
        nc.tensor.matmul(PS[:], lhsT=W[:], rhs=ACC[:], start=True, stop=True)
        FIN = pool.tile([1, 8], f32, name="FIN")
        nc.vector.tensor_copy(FIN[:], PS[:])

        nc.sync.dma_start(out=out.ap(), in_=FIN[:])


def build_bass():
    global _CACHED_NC
    if _CACHED_NC is not None:
        return _CACHED_NC
    import concourse.bacc as bacc
    import concourse.tile as tile
    import concourse.mybir as mybir
    import concourse.bass_utils as _bu

    # The NEFF epilogue zeroes the whole sem file one EVENT_SEMAPHORE at a
    # time (~6us, serialized on the slowest engine). --max-sem-num=150 trims
    # the first few walrus-internal sems out of that clear set (3..6).
    if not hasattr(_bu, "_orig_get_walrus_args"):
        _bu._orig_get_walrus_args = _bu.get_walrus_args

        def _patched_walrus_args(*a, **k):
            return [*_bu._orig_get_walrus_args(*a, **k), "--max-sem-num=150"]

        _bu.get_walrus_args = _patched_walrus_args

    f32 = mybir.dt.float32
    fp8 = mybir.dt.float8e3
    Act = mybir.ActivationFunctionType

    class FastTileContext(tile.TileContext):
        # TileContext._drain_and_barrier minus the end-of-kernel barrier and
        # the semaphore range-clear - the NEFF's own epilogue already runs a
        # pre-storm all-engine barrier, and with every kernel sem steered
        # into 207..255 the storm's Sync-engine portion (which runs after
        # Sync's drain, i.e. after the output DMA lands) covers the reset.
        def _drain_and_barrier(self, tick_clock, wait_clock):
            clock = tile.ScopedClock({None: tick_clock.global_clock})
            drain_inst = self.nc.sync.drain()
            wait_clock.add_sem_waits(drain_inst.ins, clock)
            popped = self.nc._tile_sem_poison_stack.pop()
            assert popped is self._sem_poison
            # No barrier and no explicit sem clear: the NEFF's own epilogue
            # runs a pre-storm all-engine barrier, and its Sync-engine clear
            # range (207..255) covers every sem this kernel uses.

    nc = bacc.Bacc("TRN2", target_bir_lowering=False, debug=False,
                   num_devices=NCORES)
    # Steer every tile/DMA semaphore into 207..255: keeps the live sems out
    # of the clear ranges the NEFF epilogue hands to the non-Sync engines.
    nc._state.reset_free_semaphores(list(range(207, 256)))
    big = nc.dram_tensor("big", [128, W_BIG], fp8, kind="ExternalInput")
    out = nc.dram_tensor("partials", [1, 8], f32, kind="ExternalOutput")
    with FastTileContext(nc) as tc:
        _emit(nc, tc, mybir, big, out)

    # Route every Exp/Ln to the one table that holds both, so the kernel pays
    # a single ACT_TABLE_LOAD instead of ping-ponging between per-func tables.
    orig_tables = bacc.get_activation_tables

    def _merged_tables(arch):
        out_d = {}
        for name, s in orig_tables(arch).items():
            s2 = set(s)
            if name != "natural_log_exp_and_others":
                s2.discard(Act.Exp)
                s2.discard(Act.Ln)
            out_d[name] = s2
        return out_d

    bacc.get_activation_tables = _merged_tables
    try:
        nc.compile()
    finally:
        bacc.get_activation_tables = orig_tables

    # Drop the dead const memsets (this kernel never reads them): the gpsimd
    # const chain gates the tile-enter dance, so each dead memset costs
    # ~0.1us of every core's prologue.
    entry = nc.main_func.blocks[0]
    dead_consts = ("const-bfloat16-1.0", "const-uint8-127",
                   "const-float32-0.0", "const-float32-1.0")
    entry.instructions[:] = [
        ins for ins in entry.instructions
        if not (type(ins).__name__ == "InstMemset"
                and getattr(ins, "outs", None)
                and any(d in str(ins.outs[0]) for d in dead_consts))
    ]

    # Drop a spurious default-table InstLoadActFuncSet: when two loads appear
    # with no activation between them, the first is dead and its 1.3us sits
    # right before the first Exp on the critical path.
    for blk in nc.main_func.blocks:
        loads = []
        acts_seen = set()
        for idx, ins in enumerate(blk.instructions):
            tn = type(ins).__name__
            if tn == "InstLoadActFuncSet":
                loads.append((idx, ins))
            elif tn == "InstActivation":
                acts_seen.add(len(loads))
        if len(loads) == 2 and 1 not in acts_seen and loads[0][1].sync_info is None:
            blk.instructions.pop(loads[0][0])

    _CACHED_NC = nc
    return nc


def make_in_maps(pred_bbox, pred_obj, pred_cls, gt_boxes, gt_labels):
    """Pack per-core device inputs; also return the host-side f64 tail sums
    (positive-anchor softplus and picked-logit totals)."""
    import ml_dtypes

    fp8 = ml_dtypes.float8_e3m4
    labels = np.asarray(gt_labels).astype(np.int64)
    in_maps = []
    host_tails = []
    for core in range(NCORES):
        bs = slice(core * BPC, (core + 1) * BPC)

        buf = np.zeros((128, W_BIG), np.uint8)

        po = np.asarray(pred_obj[bs], np.float32)
        buf[0:P_OBJ, 0:F_OBJ] = po.reshape(P_OBJ, F_OBJ).astype(fp8).view(np.uint8)

        cl = np.asarray(pred_cls[bs, :M], np.float32).reshape(P_CLS, W_CLS)
        buf[0:P_CLS, 800:1440] = cl.astype(fp8).view(np.uint8)

        boxes = np.empty((P_BOX, 2, NPAIR, 4), np.float32)
        pb = np.asarray(pred_bbox[bs, :M], np.float32).reshape(BPC, P_BOX, 4, 4)
        gb = np.asarray(gt_boxes[bs], np.float32).reshape(BPC, P_BOX, 4, 4)
        boxes[:, 0] = pb.transpose(1, 0, 2, 3).reshape(P_BOX, NPAIR, 4)
        boxes[:, 1] = gb.transpose(1, 0, 2, 3).reshape(P_BOX, NPAIR, 4)
        buf[0:P_BOX, 1440:1952] = boxes.reshape(P_BOX, 128).view(np.uint8)

        in_maps.append({"big": buf.view(fp8)})

        pos = po[:, :M].astype(np.float64)
        sp_neg_of_pos = np.sum(np.logaddexp(0.0, -pos))   # softplus(-x)
        sp_pos_of_pos = np.sum(np.logaddexp(0.0, pos))    # softplus(+x)
        picked = np.take_along_axis(
            np.asarray(pred_cls[bs, :M], np.float32),
            labels[bs][..., None].astype(np.int64), axis=-1,
        )[..., 0]
        host_tails.append((sp_neg_of_pos, sp_pos_of_pos,
                           float(np.sum(picked.astype(np.float64)))))
    return in_maps, host_tails


def finalize(per_core_partials, host_tails):
    s_iou = s_ratio = s_all = s_lse = 0.0
    s_pos = s_posplus = s_picked = 0.0
    for p, (t_pos, t_posplus, t_picked) in zip(per_core_partials, host_tails):
        p = np.asarray(p, np.float64).reshape(-1)
        s_iou += p[0]
        s_ratio += p[1]
        s_all += p[2]
        s_lse += p[3]
        s_pos += t_pos
        s_posplus += t_posplus
        s_picked += t_picked
    n_pos = B * M
    n_neg = B * (N - M)
    loss_bbox = 5.0 * (n_pos - s_iou + s_ratio) / n_pos
    loss_obj = s_pos / n_pos + 0.5 * (s_all - s_posplus) / n_neg
    loss_cls = (s_lse - s_picked) / n_pos
    total = loss_bbox + loss_obj + loss_cls
    return np.array([total, loss_bbox, loss_obj, loss_cls], dtype=np.float32)


def kernel(pred_bbox, pred_obj, pred_cls, gt_boxes, gt_labels):
    from concourse.bass_utils import run_bass_kernel_spmd

    nc = build_bass()
    in_maps, host_tails = make_in_maps(pred_bbox, pred_obj, pred_cls,
                                       gt_boxes, gt_labels)
    res = run_bass_kernel_spmd(nc, in_maps, core_ids=list(range(NCORES)))
    return finalize([r["partials"] for r in res.results], host_tails)


# revision 5
# speedup vs baseline: 1.0356x; 1.0356x over previous
# Trainium2 Bass kernel for nn_DetectionLoss (B=32, N=25200, M=200, C=80).
#
# Strategy: pure data-parallel over batch (4 batches per core, 8 cores).
# The reference only reads pred_bbox[:, :M] and pred_cls[:, :M], so only
# those slices are shipped to the device. Each core computes per-partition
# partial sums of the loss terms; the host does the final (tiny) cross-core
# reduction and mean/lambda arithmetic in float64. The O(B*M) tails the
# host already gathers (positive-anchor softplus, picked class logits) are
# summed on the host in float64 - the device keeps every O(B*N) / O(B*M*C)
# reduction.
#
# Device inputs per core, split across the 3 DMA-capable engine queues
# (SP / Activation / Pool) so descriptor consumption is parallel and the
# tensors land in the order the compute chain needs them (measured ~20-25ns
# per descriptor per queue + ~1.2us pickup):
#   cls  [100, 640] fp8: cls logits [p, a=8, c=80]; 2 queues x 50 rows,
#                        lands first - it gates the Scalar chain.
#   obj  [126, 800] fp8: all 4*25200 obj logits flat; 3 queues x 42 rows.
#   small[50, 512B] fp8-bytes: pred|gt boxes as RAW f32 bytes
#                        [p, s=2, j=16, c=4]; bitcast to f32 on device.
# Boxes stay f32 because the near-zero enclose/union denominators amplify
# input rounding; fp8 logits keep ~1% per-element error that averages out
# over the 6400..100800-element means; bf16 exp/ln intermediates round at
# 0.4% and also average out (measured end-to-end rel err ~3e-5).
#
# On-device: softplus as Exp -> Ln(x*1+1) (the +1 rides the Ln activation's
# pre-bias, no elementwise pass) with accum_out; lse as Exp -> DVE reduce ->
# Ln with accum; GIoU on DVE from the bitcast f32 view; ACC[128,8] collapses
# to [1,8] via a PE matmul against an all-ones column (ACC rows 126:128 stay
# zero) so the output DMA is a single descriptor.

import numpy as np

B, N, M, C = 32, 25200, 200, 80
NCORES = 8
BPC = B // NCORES          # 4 batches per core
P_OBJ, F_OBJ = 126, 800    # 4*25200 = 126*800 exactly
P_CLS, W_CLS = 100, 640    # 8 anchors x 80 classes per row
P_BOX, NPAIR = 50, 16      # box pair partitions, 16 pairs per row
EPS = 1e-7

_CACHED_NC = None


def _emit(nc, tc, mybir, obj, cls_t, small, out):
    f32 = mybir.dt.float32
    bf16 = mybir.dt.bfloat16
    Alu = mybir.AluOpType
    Act = mybir.ActivationFunctionType

    with tc.tile_pool(name="main", bufs=1) as pool:
        ACC = pool.tile([128, 8], f32, name="ACC")
        nc.vector.memset(ACC[:], 0.0)
        # Activation bias constants + the all-ones matmul column, built
        # in-block on DVE: the Bass preamble's gpsimd const memsets gate the
        # tile-enter dance, so registering our own lets the (now-dead)
        # preamble ones be stripped after compile.
        CB = pool.tile([128, 2], f32, name="CB")
        nc.vector.memset(CB[:, 0:1], 0.0)
        nc.vector.memset(CB[:, 1:2], 1.0)
        W = pool.tile([128, 1], f32, name="W")
        nc.vector.memset(W[:], 1.0)

        OBJ = pool.tile([P_OBJ, F_OBJ], mybir.dt.float8e3, name="OBJ")
        CLS = pool.tile([P_CLS, W_CLS], mybir.dt.float8e3, name="CLS")
        SM = pool.tile([P_BOX, 512], mybir.dt.float8e3, name="SM")
        # Queue rides: sync = cls half + obj third; scalar = cls half + obj
        # third; gpsimd = small + obj third. cls and small go first on their
        # rings - they gate the Scalar act chain and the DVE GIoU chain.
        nc.sync.dma_start(out=CLS[0:50], in_=cls_t.ap()[0:50])
        nc.scalar.dma_start(out=CLS[50:100], in_=cls_t.ap()[50:100])
        nc.gpsimd.dma_start(out=SM[:], in_=small.ap())
        nc.sync.dma_start(out=OBJ[0:42], in_=obj.ap()[0:42])
        nc.scalar.dma_start(out=OBJ[42:84], in_=obj.ap()[42:84])
        nc.gpsimd.dma_start(out=OBJ[84:126], in_=obj.ap()[84:126])

        # Anchor: a dependency-free 1-col Exp so insert_act_table_loads
        # places the (1.3us) ACT_TABLE_LOAD here, overlapping the input DMA
        # flight, instead of behind the input-tile sem waits.
        DUM = pool.tile([128, 1], f32, name="DUM")
        nc.scalar.activation(DUM[:], ACC[:, 7:8], Act.Exp, bias=CB[:, 0:1])

        # ---------------- classification exp (first: lands first) --------
        Ec = pool.tile([P_CLS, 8, C], bf16, name="Ec")
        nc.scalar.activation(
            Ec[:].rearrange("p a c -> p (a c)"), CLS[:], Act.Exp,
            bias=CB[0:P_CLS, 0:1],
        )

        # ---------------- objectness softplus ----------------
        # sum softplus(x): Exp on ACT, then Ln with pre-activation bias=1.0
        # (out = ln(in*1 + 1)) with accum_out - no elementwise +1 pass.
        Eo = pool.tile([P_OBJ, F_OBJ], bf16, name="Eo")
        Lg = pool.tile([P_OBJ, F_OBJ], bf16, name="Lg")
        nc.scalar.activation(Eo[:], OBJ[:], Act.Exp, bias=CB[0:P_OBJ, 0:1])
        nc.scalar.activation(Lg[:], Eo[:], Act.Ln, bias=CB[0:P_OBJ, 1:2],
                             accum_out=ACC[0:P_OBJ, 2:3])

        # ---------------- bbox GIoU term (DVE, from the bitcast view) -----
        PB = SM[:, :].bitcast(f32).rearrange("p (s j c) -> p s j c", s=2, c=4)
        cxcy = PB[:, :, :, 0:2]
        wh = PB[:, :, :, 2:4]
        C1 = pool.tile([P_BOX, 2, NPAIR, 2], f32, name="C1")
        C2 = pool.tile([P_BOX, 2, NPAIR, 2], f32, name="C2")
        nc.vector.scalar_tensor_tensor(C1[:], wh, -0.5, cxcy, Alu.mult, Alu.add)
        nc.vector.scalar_tensor_tensor(C2[:], wh, 0.5, cxcy, Alu.mult, Alu.add)
        I1 = pool.tile([P_BOX, NPAIR, 2], f32, name="I1")
        I2 = pool.tile([P_BOX, NPAIR, 2], f32, name="I2")
        E1 = pool.tile([P_BOX, NPAIR, 2], f32, name="E1")
        E2 = pool.tile([P_BOX, NPAIR, 2], f32, name="E2")
        nc.vector.tensor_tensor(I1[:], C1[:, 0], C1[:, 1], Alu.max)
        nc.vector.tensor_tensor(I2[:], C2[:, 0], C2[:, 1], Alu.min)
        nc.vector.tensor_tensor(E1[:], C1[:, 0], C1[:, 1], Alu.min)
        nc.vector.tensor_tensor(E2[:], C2[:, 0], C2[:, 1], Alu.max)
        ID = pool.tile([P_BOX, NPAIR, 2], f32, name="ID")
        IDr = pool.tile([P_BOX, NPAIR, 2], f32, name="IDr")
        ED = pool.tile([P_BOX, NPAIR, 2], f32, name="ED")
        nc.vector.tensor_sub(ID[:], I2[:], I1[:])
        nc.vector.tensor_relu(IDr[:], ID[:])
        nc.vector.tensor_sub(ED[:], E2[:], E1[:])
        inter = pool.tile([P_BOX, NPAIR], f32, name="inter")
        encl = pool.tile([P_BOX, NPAIR], f32, name="encl")
        nc.vector.tensor_mul(inter[:], IDr[:, :, 0], IDr[:, :, 1])
        nc.vector.tensor_mul(encl[:], ED[:, :, 0], ED[:, :, 1])
        A = pool.tile([P_BOX, 2, NPAIR], f32, name="A")
        nc.vector.tensor_mul(A[:], PB[:, :, :, 2], PB[:, :, :, 3])
        asum = pool.tile([P_BOX, NPAIR], f32, name="asum")
        nc.vector.tensor_add(asum[:], A[:, 0], A[:, 1])
        U = pool.tile([P_BOX, NPAIR], f32, name="U")
        nc.vector.scalar_tensor_tensor(U[:], inter[:], -1.0, asum[:],
                                       Alu.mult, Alu.add)
        # U+eps and encl+eps laid adjacently so one reciprocal covers both
        R = pool.tile([P_BOX, 2, NPAIR], f32, name="R")
        R2 = pool.tile([P_BOX, 2, NPAIR], f32, name="R2")
        nc.vector.tensor_scalar_add(R[:, 0], U[:], EPS)
        nc.vector.tensor_scalar_add(R[:, 1], encl[:], EPS)
        nc.vector.reciprocal(R2[:], R[:])
        # NOTE: tensor_tensor_reduce wedges the device (NRT_EXEC_UNIT_UNRECOVERABLE)
        # on this runtime; scalar_tensor_tensor's accum_out path works.
        t8a = pool.tile([P_BOX, NPAIR], f32, name="t8a")
        nc.vector.scalar_tensor_tensor(
            t8a[:], inter[:], 1.0, R2[:, 0], Alu.mult, Alu.mult,
            accum_out=ACC[0:P_BOX, 0:1],
        )
        EmU = pool.tile([P_BOX, NPAIR], f32, name="EmU")
        nc.vector.tensor_sub(EmU[:], encl[:], U[:])
        t8b = pool.tile([P_BOX, NPAIR], f32, name="t8b")
        nc.vector.scalar_tensor_tensor(
            t8b[:], EmU[:], 1.0, R2[:, 1], Alu.mult, Alu.mult,
            accum_out=ACC[0:P_BOX, 1:2],
        )

        # cls tail: DVE reduce after the GIoU chain, then Ln on ACT
        sums = pool.tile([P_CLS, 8], f32, name="sums")
        lse = pool.tile([P_CLS, 8], f32, name="lse")
        nc.vector.reduce_sum(out=sums[:], in_=Ec[:], axis=mybir.AxisListType.X)
        nc.scalar.activation(lse[:], sums[:], Act.Ln,
                             bias=CB[0:P_CLS, 0:1],
                             accum_out=ACC[0:P_CLS, 3:4])

        # Collapse ACC [128,8] to [1,8] on the idle PE (all-ones column; ACC
        # rows 126:128 are never written) so the output DMA is 1 descriptor.
        from concourse.bass import MemorySpace
        PS, _ps_free = tc.tile([1, 8], f32, space=MemorySpace.PSUM, name="PS")
        nc.tensor.matmul(PS[:], lhsT=W[:], rhs=ACC[:], start=True, stop=True)
        FIN = pool.tile([1, 8], f32, name="FIN")
        nc.vector.tensor_copy(FIN[:], PS[:])

        nc.sync.dma_start(out=out.ap(), in_=FIN[:])


def build_bass():
    global _CACHED_NC
    if _CACHED_NC is not None:
        return _CACHED_NC
    import concourse.bacc as bacc
    import concourse.tile as tile
    import concourse.mybir as mybir
    import concourse.bass_utils as _bu

    # The NEFF epilogue zeroes the whole sem file one EVENT_SEMAPHORE at a
    # time (~6us, serialized on the slowest engine). --max-sem-num=150 trims
    # the first few walrus-internal sems out of that clear set (3..6).
    if not hasattr(_bu, "_orig_get_walrus_args"):
        _bu._orig_get_walrus_args = _bu.get_walrus_args

        def _patched_walrus_args(*a, **k):
            return [*_bu._orig_get_walrus_args(*a, **k), "--max-sem-num=150"]

        _bu.get_walrus_args = _patched_walrus_args

    f32 = mybir.dt.float32
    fp8 = mybir.dt.float8e3
    Act = mybir.ActivationFunctionType

    class FastTileContext(tile.TileContext):
        # TileContext._drain_and_barrier minus the end-of-kernel barrier and
        # the semaphore range-clear - the NEFF's own epilogue already runs a
        # pre-storm all-engine barrier, and with every kernel sem steered
        # into 207..255 the storm's Sync-engine portion (which runs after
        # Sync's drain, i.e. after the output DMA lands) covers the reset.
        def _drain_and_barrier(self, tick_clock, wait_clock):
            clock = tile.ScopedClock({None: tick_clock.global_clock})
            drain_inst = self.nc.sync.drain()
            wait_clock.add_sem_waits(drain_inst.ins, clock)
            popped = self.nc._tile_sem_poison_stack.pop()
            assert popped is self._sem_poison
            # No barrier and no explicit sem clear: the NEFF's own epilogue
            # runs a pre-storm all-engine barrier, and its Sync-engine clear
            # range (207..255) covers every sem this kernel uses.

    nc = bacc.Bacc("TRN2", target_bir_lowering=False, debug=False,
                   num_devices=NCORES)
    # Steer every tile/DMA semaphore into 207..255: keeps the live sems out
    # of the clear ranges the NEFF epilogue hands to the non-Sync engines.
    nc._state.reset_free_semaphores(list(range(207, 256)))
    obj = nc.dram_tensor("obj", [P_OBJ, F_OBJ], fp8, kind="ExternalInput")
    cls_t = nc.dram_tensor("cls", [P_CLS, W_CLS], fp8, kind="ExternalInput")
    small = nc.dram_tensor("small", [P_BOX, 512], fp8, kind="ExternalInput")
    out = nc.dram_tensor("partials", [1, 8], f32, kind="ExternalOutput")
    with FastTileContext(nc) as tc:
        _emit(nc, tc, mybir, obj, cls_t, small, out)

    # Route every Exp/Ln to the one table that holds both, so the kernel pays
    # a single ACT_TABLE_LOAD instead of ping-ponging between per-func tables.
    orig_tables = bacc.get_activation_tables

    def _merged_tables(arch):
        out_d = {}
        for name, s in orig_tables(arch).items():
            s2 = set(s)
            if name != "natural_log_exp_and_others":
                s2.discard(Act.Exp)
                s2.discard(Act.Ln)
            out_d[name] = s2
        return out_d

    bacc.get_activation_tables = _merged_tables
    try:
        nc.compile()
    finally:
        bacc.get_activation_tables = orig_tables

    # Drop the dead const memsets (this kernel never reads them): the gpsimd
    # const chain gates the tile-enter dance, so each dead memset costs
    # ~0.1us of every core's prologue.
    entry = nc.main_func.blocks[0]
    dead_consts = ("const-bfloat16-1.0", "const-uint8-127",
                   "const-float32-0.0", "const-float32-1.0")
    entry.instructions[:] = [
        ins for ins in entry.instructions
        if not (type(ins).__name__ == "InstMemset"
                and getattr(ins, "outs", None)
                and any(d in str(ins.outs[0]) for d in dead_consts))
    ]

    # Drop a spurious default-table InstLoadActFuncSet: when two loads appear
    # with no activation between them, the first is dead and its 1.3us sits
    # right before the first Exp on the critical path.
    for blk in nc.main_func.blocks:
        loads = []
        acts_seen = set()
        for idx, ins in enumerate(blk.instructions):
            tn = type(ins).__name__
            if tn == "InstLoadActFuncSet":
                loads.append((idx, ins))
            elif tn == "InstActivation":
                acts_seen.add(len(loads))
        if len(loads) == 2 and 1 not in acts_seen and loads[0][1].sync_info is None:
            blk.instructions.pop(loads[0][0])

    _CACHED_NC = nc
    return nc


def make_in_maps(pred_bbox, pred_obj, pred_cls, gt_boxes, gt_labels):
    """Pack per-core device inputs; also return the host-side f64 tail sums
    (positive-anchor softplus and picked-logit totals)."""
    import ml_dtypes

    fp8 = ml_dtypes.float8_e3m4
    labels = np.asarray(gt_labels).astype(np.int64)
    in_maps = []
    host_tails = []
    for core in range(NCORES):
        bs = slice(core * BPC, (core + 1) * BPC)

        po = np.asarray(pred_obj[bs], np.float32)
        obj = po.reshape(P_OBJ, F_OBJ).astype(fp8)

        cl = np.asarray(pred_cls[bs, :M], np.float32).reshape(P_CLS, W_CLS)

        boxes = np.empty((P_BOX, 2, NPAIR, 4), np.float32)
        pb = np.asarray(pred_bbox[bs, :M], np.float32).reshape(BPC, P_BOX, 4, 4)
        gb = np.asarray(gt_boxes[bs], np.float32).reshape(BPC, P_BOX, 4, 4)
        boxes[:, 0] = pb.transpose(1, 0, 2, 3).reshape(P_BOX, NPAIR, 4)
        boxes[:, 1] = gb.transpose(1, 0, 2, 3).reshape(P_BOX, NPAIR, 4)

        in_maps.append({
            "obj": obj,
            "cls": cl.astype(fp8),
            "small": boxes.reshape(P_BOX, 128).view(np.uint8).view(fp8),
        })

        pos = po[:, :M].astype(np.float64)
        picked = np.take_along_axis(
            np.asarray(pred_cls[bs, :M], np.float32),
            labels[bs][..., None].astype(np.int64), axis=-1,
        )[..., 0]
        host_tails.append((
            float(np.sum(np.logaddexp(0.0, -pos))),   # softplus(-pos)
            float(np.sum(np.logaddexp(0.0, pos))),    # softplus(+pos)
            float(np.sum(picked.astype(np.float64))),
        ))
    return in_maps, host_tails


def finalize(per_core_partials, host_tails):
    s_iou = s_ratio = s_all = s_lse = 0.0
    s_pos = s_posplus = s_picked = 0.0
    for p, (t_pos, t_posplus, t_picked) in zip(per_core_partials, host_tails):
        p = np.asarray(p, np.float64).reshape(-1)
        s_iou += p[0]
        s_ratio += p[1]
        s_all += p[2]
        s_lse += p[3]
        s_pos += t_pos
        s_posplus += t_posplus
        s_picked += t_picked
    n_pos = B * M
    n_neg = B * (N - M)
    loss_bbox = 5.0 * (n_pos - s_iou + s_ratio) / n_pos
    loss_obj = s_pos / n_pos + 0.5 * (s_all - s_posplus) / n_neg
    loss_cls = (s_lse - s_picked) / n_pos
    total = loss_bbox + loss_obj + loss_cls
    return np.array([total, loss_bbox, loss_obj, loss_cls], dtype=np.float32)


def kernel(pred_bbox, pred_obj, pred_cls, gt_boxes, gt_labels):
    from concourse.bass_utils import run_bass_kernel_spmd

    nc = build_bass()
    in_maps, host_tails = make_in_maps(pred_bbox, pred_obj, pred_cls,
                                       gt_boxes, gt_labels)
    res = run_bass_kernel_spmd(nc, in_maps, core_ids=list(range(NCORES)))
    return finalize([r["partials"] for r in res.results], host_tails)
